# revision 1
# baseline (speedup 1.0000x reference)
"""BinaryConv2d (sign-binarized 3x3 conv, B=32 C=64->64 H=W=224, pad 1) on 8
Trainium2 NeuronCores.

Strategy (data-parallel): shard the batch of 32 images across 8 cores (4
images each); the tiny binarized weight/bias are replicated. Each core runs
an identical Bass/Tile program on its shard; outputs are concatenated.

Per-core kernel:
  Images are processed in pairs so SBUF partitions = (img_in_pair*64 + ci)
  use the full 128-partition width. Activations are cast fp32->bf16 in-flight
  by SWDGE DMA into a row ring buffer XB [128, (H+2)*(W+1)+1]: slot s holds
  image row s-1 at cols [s*PITCH+1, s*PITCH+1+W), col s*PITCH is a shared
  zero pad (doubles as right pad of slot s-1); slots 0 and H+1 are permanent
  zero rows. The 3x3 conv is 9 PSUM-accumulated matmuls (K=64 channels,
  M=64 couts, binarized +-1 bf16 weights), run 4-way concurrently on the
  PE's 64x64 quadrants via tile positions (rhs partition half x psum
  partition half). A superstep covers 4 output rows x 2 images with 2 PSUM
  banks; ScalarE and VectorE drain the banks with the bias add fused;
  staged fp32 results are DMA'd out in large strided transfers.
"""

import numpy as np
import ml_dtypes

import concourse.bass as bass
import concourse.mybir as mybir
import concourse.tile as tile
from concourse import bacc
from concourse.bass_utils import run_bass_kernel_spmd
from contextlib import ExitStack

F32 = mybir.dt.float32
BF16 = mybir.dt.bfloat16
AFT = mybir.ActivationFunctionType

B, CIN, COUT, H, W = 32, 64, 64, 224, 224
N_CORES = 8
NIMG = B // N_CORES  # images per core


def _pack_weights(weight: np.ndarray) -> np.ndarray:
    wb = np.sign(weight.astype(np.float32))
    wt = wb.transpose(1, 2, 3, 0).reshape(CIN, 9 * COUT)
    full = np.concatenate([wt, wt], axis=0).astype(ml_dtypes.bfloat16)
    return np.ascontiguousarray(full)


def _pack_bias(bias: np.ndarray) -> np.ndarray:
    b = bias.astype(np.float32).reshape(COUT, 1)
    return np.ascontiguousarray(np.concatenate([b, b], axis=0))


def build_conv_nc(nimg: int = NIMG, ss_per_flush: int = 8, slab: int = 16,
                  lookahead_slabs: int = 2, psum_bufs: int = 4,
                  loop_iters: int = 1, out_split: bool = True,
                  inter: bool = True, absorbers: bool = True,
                  out_eng_sync: bool = False, hw_in: bool = False):
    assert nimg % 2 == 0
    npair = nimg // 2
    PITCH = W + 1
    R = H + 2
    n_ss = H // 4

    nc = bacc.Bacc("TRN2", target_bir_lowering=False, debug=False)
    x_in = nc.dram_tensor("x", [nimg * 64, H, W], F32, kind="ExternalInput")
    wt_in = nc.dram_tensor("wt", [128, 9 * COUT], BF16, kind="ExternalInput")
    b_in = nc.dram_tensor("bias", [128, 1], F32, kind="ExternalInput")
    out = nc.dram_tensor("out", [nimg * 64, H, W], F32, kind="ExternalOutput")

    xflat = x_in.rearrange("p h w -> p (h w)")
    oflat = out.rearrange("p h w -> p (h w)")

    with tile.TileContext(nc) as tc, ExitStack() as ctx:
        const_pool = ctx.enter_context(tc.tile_pool(name="const", bufs=1))
        psum_pool = ctx.enter_context(
            tc.tile_pool(name="psum", bufs=psum_bufs, space="PSUM"))
        sga_pool = ctx.enter_context(tc.tile_pool(name="sga", bufs=2))
        sgb_pool = ctx.enter_context(tc.tile_pool(name="sgb", bufs=2))
        land_pool = ctx.enter_context(tc.tile_pool(name="land", bufs=2))

        XB = const_pool.tile([128, R * PITCH + 1], BF16)
        WT = const_pool.tile([128, 9 * COUT], BF16)
        BI = const_pool.tile([128, 1], F32)

        nc.sync.dma_start(WT[:, :], wt_in[:, :])
        nc.sync.dma_start(BI[:, :], b_in[:, :])
        # Zero the whole ring once: pads + permanent zero rows. Slab DMAs get
        # WAW deps on this, so the waits live on the DMA queue.
        nc.vector.memset(XB[:, :], 0.0)

        xb_flat = XB[:, :]
        xb_pstep = xb_flat.ap[0][0]
        xb_off0 = xb_flat.offset
        xb_slots = XB[:, : R * PITCH].rearrange("p (s c) -> p s c", c=PITCH)

        def rhs_ap(part0, col0, nrows):
            dims = [(xb_pstep, 64)]
            if nrows > 1:
                dims.append((PITCH, nrows))
            dims.append((1, W))
            return bass.AP(tensor=xb_flat.tensor,
                           offset=xb_off0 + part0 * xb_pstep + col0, ap=dims)

        def absorb(col0, ncols=1):
            """Token ldweights reading XB cols [col0,col0+ncols): new-data
            sync waits land on an InstLdweights (junk weights, overwritten
            by each matmul's own weight load)."""
            ap = bass.AP(tensor=xb_flat.tensor, offset=xb_off0 + col0,
                         ap=[(xb_pstep, 64), (1, ncols)])
            nc.tensor.ldweights(weights=ap)

        nc.tensor.ldweights(weights=WT[0:64, 0:1])
        SCR = const_pool.tile([128, 2], F32)
        nc.scalar.activation(SCR[:, 0:1], BI[:, :], AFT.Identity)
        nc.vector.tensor_scalar_add(SCR[:, 1:2], BI[:, :], 0.0)

        def issue_in_slab(p, k):
            r0 = k * slab
            nrows = min(slab, H - r0)
            src = xflat[2 * p * 64 : (2 * p + 2) * 64,
                        r0 * W : (r0 + nrows) * W]
            dst = xb_slots[:, 1 + r0 : 1 + r0 + nrows, 1 : 1 + W]
            if hw_in:
                # HWDGE fp32 load to a landing tile, then a VectorE
                # narrowing copy into the bf16 ring (bypasses SWDGE cast).
                LD = land_pool.tile([128, slab * W], F32, tag="land")
                nc.sync.dma_start(out=LD[:, 0 : nrows * W], in_=src)
                l3 = LD[:, 0 : nrows * W].rearrange("q (r w) -> q r w", w=W)
                nc.vector.tensor_copy(dst, l3)
            else:
                src3 = src.rearrange("q (r w) -> q r w", w=W)
                nc.gpsimd.dma_start(out=dst, in_=src3)

        n_slabs = (H + slab - 1) // slab

        def emit_all():
          for p in range(npair):
            slabs_issued = [0]

            def ensure_slabs(upto_slot_incl, p=p, s_i=slabs_issued):
                need = min(n_slabs,
                           max(0, upto_slot_incl - 1) // slab + 1 + lookahead_slabs)
                while s_i[0] < need:
                    issue_in_slab(p, s_i[0])
                    s_i[0] += 1

            seen_hi = [-1]
            for fl0 in range(0, n_ss, ss_per_flush):
                fl_n = min(ss_per_flush, n_ss - fl0)
                SGA = sga_pool.tile([128, ss_per_flush * 2 * W], F32, tag="sga")
                SGB = sgb_pool.tile([128, ss_per_flush * 2 * W], F32, tag="sgb")
                # Token writes absorb the staging-slot WAR wait (previous
                # flush's out-DMA) so drains only wait on the PE.
                nc.scalar.activation(SGA[:, 0:1], BI[:, :], AFT.Identity)
                nc.vector.tensor_scalar_add(SGB[:, 0:1], BI[:, :], 0.0)
                h0 = fl0 * 4
                ro_hi = 2 * fl_n  # psum half [64:128] takes the flush's
                # second half of rows, so each partition's rows come out
                # contiguous and the flush is one 3-dim full-partition DMA.
                for sl in range(fl_n):
                    if inter:
                        a = h0 + 4 * sl
                        c = a + 2
                    else:
                        a = h0 + 2 * sl      # rows for psum half [0:64]
                        c = a + ro_hi        # rows for psum half [64:128]
                    hi = min(c + 3, H + 1)
                    ensure_slabs(hi)
                    if absorbers:
                        for s in range(max(0, seen_hi[0] + 1), hi + 1):
                            absorb(s * PITCH + 1)
                        seen_hi[0] = max(seen_hi[0], hi)

                    PA = psum_pool.tile([128, 2 * W], F32, tag="ps")
                    PB = psum_pool.tile([128, 2 * W], F32, tag="ps")
                    for tap in range(9):
                        kh, kw = divmod(tap, 3)
                        first, last = tap == 0, tap == 8
                        for ih, P, pc, rb in ((0, PA, 0, a), (64, PB, 0, a),
                                              (0, PA, 64, c), (64, PB, 64, c)):
                            s0 = rb + kh
                            lhsT = WT[ih : ih + 64, tap * 64 : (tap + 1) * 64]
                            nc.tensor.matmul(P[pc : pc + 64, 0 : 2 * W], lhsT,
                                             rhs_ap(ih, s0 * PITCH + kw, 2),
                                             start=first, stop=last,
                                             skip_group_check=True)
                    c0 = sl * 2 * W
                    nc.scalar.activation(SGA[:, c0 : c0 + 2 * W], PA[:, :],
                                         AFT.Identity, bias=BI[:, :])
                    nc.vector.tensor_scalar_add(SGB[:, c0 : c0 + 2 * W],
                                                PB[:, :], BI[:, :])
                for (SG, img) in ((SGA, 0), (SGB, 1)):
                    pl0 = (2 * p + img) * 64
                    eng = nc.sync if (img == 0 or out_eng_sync) else nc.scalar
                    if inter:
                        for gh in range(2):
                            s2 = SG[gh * 64 : (gh + 1) * 64, 0 : fl_n * 2 * W]
                            src4 = s2.rearrange("c (s j) -> c s j", j=2 * W)
                            d2 = oflat[pl0 : pl0 + 64,
                                       h0 * W : (h0 + 4 * fl_n) * W]
                            dst4 = d2.rearrange("c (s q) -> c s q", q=4 * W)
                            dst4 = dst4[:, :, 2 * gh * W : (2 * gh + 2) * W]
                            eng.dma_start(out=dst4, in_=src4)
                    elif out_split:
                        # Two half-partition DMAs; each partition's data is
                        # one contiguous 2*fl_n-row run in HBM.
                        for g in range(2):
                            src = SG[g * 64 : (g + 1) * 64, 0 : fl_n * 2 * W]
                            r0 = h0 + g * 2 * fl_n
                            dst = oflat[pl0 : pl0 + 64,
                                        r0 * W : (r0 + 2 * fl_n) * W]
                            eng.dma_start(out=dst, in_=src)
                    else:
                        src = SG[:, 0 : fl_n * 2 * W]
                        # partition p=(g*64+co) -> plane pl0+co, contiguous
                        # rows [h0 + g*2*fl_n, h0 + (g+1)*2*fl_n)
                        dst = bass.AP(
                            tensor=oflat.tensor,
                            offset=pl0 * H * W + h0 * W,
                            ap=[(2 * fl_n * W, 2), (H * W, 64),
                                (1, 2 * fl_n * W)])
                        eng.dma_start(out=dst, in_=src)

        if loop_iters > 1:
            with tc.For_i(0, loop_iters, 1):
                emit_all()
        else:
            emit_all()
    nc.compile()
    return nc


_NC_CACHE = {}


def _get_nc():
    if "nc" not in _NC_CACHE:
        _NC_CACHE["nc"] = build_conv_nc()
    return _NC_CACHE["nc"]


def run_sharded(x, weight, bias, nc=None, **run_kwargs):
    """x [32,64,224,224] f32 -> out [32,64,224,224] f32 on 8 cores.
    Returns (out, BassKernelResults)."""
    x = np.ascontiguousarray(np.asarray(x, dtype=np.float32))
    wt = _pack_weights(np.asarray(weight))
    bi = _pack_bias(np.asarray(bias))
    if nc is None:
        nc = _get_nc()
    in_maps = []
    for i in range(N_CORES):
        xs = x[i * NIMG : (i + 1) * NIMG].reshape(NIMG * 64, H, W)
        in_maps.append({"x": np.ascontiguousarray(xs), "wt": wt, "bias": bi})
    res = run_bass_kernel_spmd(nc, in_maps, core_ids=list(range(N_CORES)),
                               **run_kwargs)
    out = np.concatenate(
        [r["out"].reshape(NIMG, COUT, H, W) for r in res.results], axis=0)
    return out, res


def kernel(x, weight, bias):
    out, _ = run_sharded(x, weight, bias)
    return out

